# revision 13
# baseline (speedup 1.0000x reference)
"""Distributed causal-self-attention kernel for one TRN2 chip (8 NeuronCores).

Reference math (T = D = N = 4096, faithful to the oracle):
    q = x @ Wq + bq ; k = x @ Wk + bk ; v = x @ Wv + bv      # [T, D]
    scores = (q @ k.T) / sqrt(D)                             # [T, T]
    p = softmax(scores, axis=-1)
    out = p @ v.T            # i.e. out[i, j] = sum_k p[i, k] * v[j, k]

Distribution
------------
Phase 1 (projections) is TENSOR-parallel: core c owns the d-slice
Dc = [512c, 512(c+1)) of all three projections and computes
qT/kT/vT[Dc, :] for ALL 4096 rows, using only its 1/8 slice of each
weight matrix plus one streamed pass over the full xT. This keeps the
per-core HBM rate in phase 1 under the chip's power-throttle knee
(~1.0 TB/s aggregate): with the sequence-parallel layout every core
reads all 100 MB of weights and the GPIO power throttle drops the PE
clock from 2.4 to 1.95 GHz for the whole kernel. Unthrottled matmuls
issue at ~226 ns (N=512 bf16); throttled at ~263 ns.

The slices are exchanged with PIECEWISE AllGathers (issued as each
1024-column piece completes) so the gathers fully overlap projection
compute: kTg/vTg pieces hold [4096 d, 1024 keys]; qT is gathered
per-512-row block and core c reads back exactly block c (its own rows)
for phase 2. Per-jb matrix order is q,k,v so the last Q gather (the
only phase-2 start dependency besides K piece 0) completes with zero
tail.

Phases 2/3 are sequence-parallel as before: core c computes scoresT/E
and the output for its own 512 rows against the full gathered K/V
(those phases inherently stream 67 MB/core and stay throttled; their
matmul stream is already at that floor). Compute is bf16 with fp32
PSUM accumulation (end-to-end rel err ~5e-3 vs the fp32 oracle).

    scoresT tile [j,i] = kT_chunk.T @ qT_chunk   (keys j on partitions)
    E = exp(scoresT / 64)        (scores are ~N(0,1); no max-subtraction)
    sums[i] = sum_j E[j, i]      (matmul with a ones vector, issued after
                                  phase 3's first block so the PE never
                                  idles at the phase boundary)
    out tile [i, jout] = sum_k E[k, i] * vT[k, jout], scaled by 1/sums[i]
"""

import os
import sys

import numpy as np

for _p in ("/opt/trn_rl_repo", "/root/.axon_site/_ro/trn_rl_repo"):
    if os.path.isdir(_p) and _p not in sys.path:
        sys.path.insert(0, _p)

import ml_dtypes

P = 128                 # partitions
T = 4096                # seq len == d == input feature dim
NCORES = 8
S = T // NCORES         # 512-wide d-slice per core / rows per core
KO = T // P             # 32 contraction chunks of 128
NB = T // S             # 8 key/value/row blocks of 512
NSUB = S // P           # 4 subtiles of 128 per 512 block
NDT = S // P            # 4 d-tiles of 128 in a core's d-slice
NPC = NB // 2           # 4 two-block gather pieces for K/V
SCALE = 1.0 / 64.0      # 1/sqrt(4096)

_BF16 = ml_dtypes.bfloat16


def _build_program():
    import concourse.mybir as mybir
    from concourse import bacc
    from concourse.tile import TileContext

    f32 = mybir.dt.float32
    bf16 = mybir.dt.bfloat16
    Ident = mybir.ActivationFunctionType.Identity
    Exp = mybir.ActivationFunctionType.Exp

    nc = bacc.Bacc(
        "TRN2",
        target_bir_lowering=False,
        debug=False,
        enable_asserts=False,
        num_devices=NCORES,
    )

    # xT is the FULL transposed input (same array on every core).
    # Ws packs this core's d-slices of the three weights, m-major in the
    # order (q, k, v): Ws[m*4+dt, p, ko*128+dd] =
    #   W_m[ko*128 + p, c*512 + dt*128 + dd].
    # b3 packs the matching bias slices: b3[p, m*4+dt] = b_m[c*512+dt*128+p].
    xT = nc.dram_tensor("xT", [T, T], bf16, kind="ExternalInput")
    Ws = nc.dram_tensor("Ws", [3 * NDT, P, T], bf16, kind="ExternalInput")
    b3 = nc.dram_tensor("b3", [P, 3 * NDT], f32, kind="ExternalInput")
    out = nc.dram_tensor("out", [S, T], f32, kind="ExternalOutput")

    rg = [list(range(NCORES))]

    with TileContext(nc) as tc:
        with tc.tile_pool(name="dram", bufs=1, space="DRAM") as dram:
            # local bounce pieces (collective inputs must be Local)
            kB = [dram.tile([S, 2 * S], bf16, name=f"kB{i}")
                  for i in range(NPC)]
            vB = [dram.tile([S, 2 * S], bf16, name=f"vB{i}")
                  for i in range(NPC)]
            # Q exchange is an AllToAll: qB chunk jb (rows [jb*512,...)) is
            # this core's d-slice of qT for rows-block jb; the output qTr
            # chunk s is core s's d-slice for OUR rows -> qTr = qT[:, ours].
            qB = dram.tile([T, S], bf16)
            qTr = dram.tile([T, S], bf16)
            # gathered pieces: kTg[pj][d, kk] = kT[d, pj*1024 + kk], etc.
            kTg = [dram.tile([T, 2 * S], bf16, addr_space="Shared",
                             name=f"kTg{i}") for i in range(NPC)]
            vTg = [dram.tile([T, 2 * S], bf16, addr_space="Shared",
                             name=f"vTg{i}") for i in range(NPC)]

            with tc.tile_pool(name="persist", bufs=1) as persist:
                ones_sb = persist.tile([P, 1], f32)
                b3_sb = persist.tile([P, 3 * NDT], f32)
                recip_sb = persist.tile([P, NSUB], f32)
                acc_sb = persist.tile([P, S], f32)
                nc.vector.memset(ones_sb[:], 1.0)

                # ---------- Phase 1: TP projections of qT, kT, vT ----------
                with tc.tile_pool(name="wsp", bufs=1) as wsp, \
                     tc.tile_pool(name="xjp", bufs=2) as xjp, \
                     tc.tile_pool(name="stage", bufs=8) as stage, \
                     tc.tile_pool(name="ppsum", bufs=8, space="PSUM") as ppsum:
                    ws_sb = wsp.tile([P, 3 * NDT, T], bf16)
                    # q/dt0 weights first (in 4 chunks so the first matmul
                    # can start ~1.5us in), then the rest; bias alongside.
                    for c4 in range(4):
                        nc.sync.dma_start(
                            ws_sb[:, 0, c4 * T // 4:(c4 + 1) * T // 4],
                            Ws[0][:, c4 * T // 4:(c4 + 1) * T // 4])
                    nc.sync.dma_start(b3_sb[:], b3[:])
                    for md in range(1, 3 * NDT):
                        nc.sync.dma_start(ws_sb[:, md, :], Ws[md])

                    xr = xT[:].rearrange("(ko p) f -> p ko f", p=P)
                    for jb in range(NB):
                        xj = xjp.tile([P, KO, S], bf16, tag="xj")
                        jlo = jb * S
                        if jb == 0:
                            # stream jb0 chunk-by-chunk in consumption order
                            # on the gpsimd queue (sync carries the weights)
                            nc.gpsimd.dma_start(
                                xj[:, 0, :], xr[:, 0, jlo:jlo + S])
                            nc.gpsimd.dma_start(
                                xj[:, 1, :], xr[:, 1, jlo:jlo + S])
                            for lo in range(2, KO, 2):
                                nc.gpsimd.dma_start(
                                    xj[:, lo:lo + 2, :],
                                    xr[:, lo:lo + 2, jlo:jlo + S])
                        else:
                            for i4 in range(4):
                                nc.sync.dma_start(
                                    xj[:, i4 * 8:(i4 + 1) * 8, :],
                                    xr[:, i4 * 8:(i4 + 1) * 8, jlo:jlo + S])
                        for m in range(3):          # q, k, v
                            for dt in range(NDT):
                                md = m * NDT + dt
                                ps = ppsum.tile([P, S], f32, tag="pp")
                                for ko in range(KO):
                                    nc.tensor.matmul(
                                        ps[:],
                                        ws_sb[:, md, ko * P:(ko + 1) * P],
                                        xj[:, ko, :],
                                        start=(ko == 0),
                                        stop=(ko == KO - 1),
                                    )
                                st = stage.tile([P, S], bf16, tag="st")
                                nc.scalar.activation(
                                    st[:], ps[:], Ident,
                                    bias=b3_sb[:, md:md + 1])
                                drow = slice(dt * P, (dt + 1) * P)
                                if m == 0:
                                    nc.sync.dma_start(
                                        qB[jb * S + dt * P:jb * S + (dt + 1) * P, :],
                                        st[:])
                                elif m == 1:
                                    nc.sync.dma_start(
                                        kB[jb // 2][drow,
                                                    (jb % 2) * S:(jb % 2 + 1) * S],
                                        st[:])
                                else:
                                    nc.sync.dma_start(
                                        vB[jb // 2][drow,
                                                    (jb % 2) * S:(jb % 2 + 1) * S],
                                        st[:])
                            if m == 0 and jb == NB - 1:
                                # all q pieces staged: exchange. Issued here
                                # (before jb7's k/v compute) so it finishes
                                # well before phase 2 starts.
                                nc.gpsimd.collective_compute(
                                    "AllToAll", mybir.AluOpType.bypass,
                                    replica_groups=rg, ins=[qB[:]],
                                    outs=[qTr[:]],
                                )
                        # piecewise k/v gathers per two blocks, overlapping
                        # the remaining projection compute
                        if jb % 2 == 1:
                            pj = jb // 2
                            nc.gpsimd.collective_compute(
                                "AllGather", mybir.AluOpType.bypass,
                                replica_groups=rg, ins=[kB[pj][:]],
                                outs=[kTg[pj][:]],
                            )
                            nc.gpsimd.collective_compute(
                                "AllGather", mybir.AluOpType.bypass,
                                replica_groups=rg, ins=[vB[pj][:]],
                                outs=[vTg[pj][:]],
                            )

                # ---------- Phase 2: scoresT -> E = exp(scoresT/64) ----------
                with tc.tile_pool(name="qTp", bufs=1) as qTp, \
                     tc.tile_pool(name="Ep", bufs=1) as Ep, \
                     tc.tile_pool(name="blocks", bufs=3) as bpool:
                    # E_sb[p, jo, i] = exp(scores[i_global, jo*128 + p] / 64)
                    E_sb = Ep.tile([P, KO, S], bf16)
                    qT_sb = qTp.tile([P, KO, S], bf16)
                    # this core's own-row qT block, in chunk-consumption order
                    qsrc = qTr[:].rearrange("(ko p) f -> p ko f", p=P)
                    nc.sync.dma_start(qT_sb[:, 0, :], qsrc[:, 0, :])
                    nc.sync.dma_start(qT_sb[:, 1, :], qsrc[:, 1, :])
                    for lo in range(2, KO, 2):
                        nc.sync.dma_start(
                            qT_sb[:, lo:lo + 2, :], qsrc[:, lo:lo + 2, :])

                    with tc.tile_pool(name="qkpsum", bufs=8, space="PSUM") as qkpsum:
                        for jb in range(NB):
                            kb = bpool.tile([P, KO, S], bf16, tag="blk")
                            src = kTg[jb // 2][:, (jb % 2) * S:(jb % 2 + 1) * S]
                            src = src.rearrange("(ko p) f -> p ko f", p=P)
                            if jb == 0:
                                nc.sync.dma_start(kb[:, 0, :], src[:, 0, :])
                                nc.sync.dma_start(kb[:, 1, :], src[:, 1, :])
                                for lo in range(2, KO, 2):
                                    nc.sync.dma_start(
                                        kb[:, lo:lo + 2, :], src[:, lo:lo + 2, :])
                            else:
                                for i4 in range(4):
                                    nc.sync.dma_start(
                                        kb[:, i4 * 8:(i4 + 1) * 8, :],
                                        src[:, i4 * 8:(i4 + 1) * 8, :],
                                    )
                            for js in range(NSUB):
                                ps = qkpsum.tile([P, S], f32, tag="qk")
                                for ko in range(KO):
                                    nc.tensor.matmul(
                                        ps[:],
                                        kb[:, ko, js * P:(js + 1) * P],
                                        qT_sb[:, ko, :],
                                        start=(ko == 0),
                                        stop=(ko == KO - 1),
                                    )
                                nc.scalar.activation(
                                    E_sb[:, jb * NSUB + js, :], ps[:], Exp,
                                    scale=SCALE)
                            Evb = E_sb[:, jb * NSUB:(jb + 1) * NSUB, :].rearrange(
                                "p ko i -> p i ko")
                            if jb == 0:
                                nc.vector.reduce_sum(
                                    acc_sb[:], Evb, axis=mybir.AxisListType.X)
                            else:
                                pt = bpool.tile([P, S], f32, tag="pt", bufs=2)
                                nc.vector.reduce_sum(
                                    pt[:], Evb, axis=mybir.AxisListType.X)
                                nc.vector.tensor_add(acc_sb[:], acc_sb[:], pt[:])

                    # ---------- Phase 3: out = (E.T @ vT) / sums ----------
                    # The softmax-denominator matmuls are issued AFTER vb=0's
                    # accumulation groups: the PE queue is strict program
                    # order and the denominator chain depends on the full DVE
                    # reduce of E — putting it first would idle the PE ~3us
                    # at the phase boundary (and re-throttle HAM). The vb=0
                    # output scaling only needs recip at its
                    # tensor_scalar_mul, well after the sums complete.
                    with tc.tile_pool(name="pvpsum", bufs=6, space="PSUM") as pvpsum, \
                         tc.tile_pool(name="spsum", bufs=2, space="PSUM") as spsum, \
                         tc.tile_pool(name="ostage", bufs=4) as ostage:
                        def self_scale_store(ps, ii, vb):
                            ot = ostage.tile([P, S], f32, tag="ot")
                            r = recip_sb[:, ii:ii + 1]
                            orow = out[ii * P:(ii + 1) * P, :]
                            if vb == NB - 1 and ii == NSUB - 1:
                                # last tile: chunk the scale+store so the
                                # final DMA starts as early as possible
                                for c in range(4):
                                    lo, hi = c * (S // 4), (c + 1) * (S // 4)
                                    nc.vector.tensor_scalar_mul(
                                        ot[:, lo:hi], ps[:, lo:hi], r)
                                    nc.sync.dma_start(
                                        orow[:, vb * S + lo:vb * S + hi],
                                        ot[:, lo:hi])
                            elif vb == NB - 1:
                                nc.vector.tensor_scalar_mul(ot[:], ps[:], r)
                                h = S // 2
                                nc.sync.dma_start(
                                    orow[:, vb * S:vb * S + h], ot[:, :h])
                                nc.sync.dma_start(
                                    orow[:, vb * S + h:(vb + 1) * S], ot[:, h:])
                            else:
                                nc.vector.tensor_scalar_mul(ot[:], ps[:], r)
                                nc.sync.dma_start(
                                    orow[:, vb * S:(vb + 1) * S], ot[:])

                        for vb in range(NB):
                            vbt = bpool.tile([P, KO, S], bf16, tag="blk")
                            src = vTg[vb // 2][:, (vb % 2) * S:(vb % 2 + 1) * S]
                            src = src.rearrange("(ko p) f -> p ko f", p=P)
                            if vb == 0:
                                nc.sync.dma_start(vbt[:, 0, :], src[:, 0, :])
                                nc.sync.dma_start(vbt[:, 1, :], src[:, 1, :])
                                for lo in range(2, KO, 2):
                                    nc.sync.dma_start(
                                        vbt[:, lo:lo + 2, :], src[:, lo:lo + 2, :])
                            else:
                                for i4 in range(4):
                                    nc.sync.dma_start(
                                        vbt[:, i4 * 8:(i4 + 1) * 8, :],
                                        src[:, i4 * 8:(i4 + 1) * 8, :],
                                    )
                            pss = []
                            for ii in range(NSUB):
                                ps = pvpsum.tile([P, S], f32, tag="pv")
                                for ko in range(KO):
                                    nc.tensor.matmul(
                                        ps[:],
                                        E_sb[:, ko, ii * P:(ii + 1) * P],
                                        vbt[:, ko, :],
                                        start=(ko == 0),
                                        stop=(ko == KO - 1),
                                    )
                                if vb == 0:
                                    pss.append(ps)
                                    continue
                                self_scale_store(ps, ii, vb)
                            if vb == 0:
                                for ii in range(NSUB):
                                    sp = spsum.tile([P, 1], f32, tag="sum")
                                    nc.tensor.matmul(
                                        sp[:], acc_sb[:, ii * P:(ii + 1) * P],
                                        ones_sb[:], start=True, stop=True)
                                    nc.vector.reciprocal(
                                        recip_sb[:, ii:ii + 1], sp[:])
                                for ii in range(NSUB):
                                    self_scale_store(pss[ii], ii, vb)
    nc.compile()
    return nc


def _tile_weight(W):
    # W_t[dtg, p, ko*128 + f] = W[ko*128 + p, dtg*128 + f]
    W4 = np.asarray(W, dtype=np.float32).reshape(KO, P, KO, P)
    return np.ascontiguousarray(W4.transpose(2, 1, 0, 3).reshape(KO, P, T)).astype(_BF16)


def _prepare_in_maps(inputs):
    x = np.asarray(inputs["x"], dtype=np.float32)
    xTf = np.ascontiguousarray(x.T).astype(_BF16)
    Wt = [_tile_weight(inputs[k]) for k in ("Wq", "Wk", "Wv")]
    bs = [np.asarray(inputs[k], np.float32) for k in ("bq", "bk", "bv")]
    in_maps = []
    for c in range(NCORES):
        Ws_c = np.ascontiguousarray(
            np.concatenate([W[c * NDT:(c + 1) * NDT] for W in Wt], axis=0))
        b3_c = np.ascontiguousarray(np.stack(
            [b[c * S + dt * P:c * S + (dt + 1) * P]
             for b in bs for dt in range(NDT)], axis=1))
        in_maps.append({"xT": xTf, "Ws": Ws_c, "b3": b3_c})
    return in_maps


def _run(inputs, trace=False, **spmd_kwargs):
    from concourse.bass_utils import run_bass_kernel_spmd

    nc = _build_program()
    in_maps = _prepare_in_maps(inputs)
    res = run_bass_kernel_spmd(
        nc, in_maps, list(range(NCORES)), trace=trace, **spmd_kwargs)
    out = np.concatenate(
        [np.asarray(res.results[c]["out"], dtype=np.float32) for c in range(NCORES)],
        axis=0,
    )
    return out, res


def kernel(**inputs):
    out, _ = _run(inputs, trace=False)
    return out
